# revision 9
# baseline (speedup 1.0000x reference)
"""LoRA linear layer on 8 Trainium2 NeuronCores.

Computes out = x @ (lora_B @ lora_A * 2).T + bias for
x [4, 2048, 4096], lora_A [16, 4096], lora_B [4096, 16], bias [4096].

Strategy: pure data parallel — shard x over batch*seq (8192 rows -> 1024
rows/core), replicate the tiny LoRA weights. Per core, exploit the rank-16
structure: y = x @ A^T (contract 4096), z = y @ B^T * 2 + bias (contract 16).

v6 (memory-regime tuning):
  * All device-side matmul traffic is bf16 (1 PE cycle/row vs 4 for fp32;
    rel-err budget is 2e-2, bf16 lands ~5e-3). Output stored bf16 and
    upcast on the host. Per-core HBM traffic: 8 MiB in + 8 MiB out.
  * Host pre-transposes each x shard to x^T and pre-tiles it so every
    1 MiB load piece is a fully contiguous [128, 4096] DRAM slab
    (8 KiB per partition line -> line-rate descriptors). No PE
    transposes, no transpose PSUM round-trip.
  * Loads are issued on the SP HWDGE ring in exact consume order;
    stores + const loads go on the separate ACT ring so they never
    head-of-line-block a load issue. SDMA engines round-robin between
    in-flight transfers, so the xin pool depth (3) bounds sharing.
  * Three row-blocks (512 + 256 + 256): a block's GEMM2/stores need the
    whole block loaded, so a smaller final block starts its stores (the
    non-overlappable tail) much earlier.
  * PSUM -> SBUF drain of z alternates Vector/Scalar, 5 z banks, so
    GEMM2 rarely stalls on bank reuse; HAM warm-up matmuls during the
    first load bring the PE to full clock before real work.
"""

import sys

import numpy as np

if "/opt/trn_rl_repo" not in sys.path:
    sys.path.insert(0, "/opt/trn_rl_repo")

import ml_dtypes

import concourse.bass as bass
import concourse.mybir as mybir
from concourse import bacc
from concourse.bass_utils import run_bass_kernel_spmd
from concourse.tile import TileContext

N_CORES = 8
B, S, IN_F, OUT_F, R = 4, 2048, 4096, 4096, 16
ROWS = B * S // N_CORES  # 1024 rows per core
SCALING = 2.0  # alpha / r = 32 / 16
FP32 = mybir.dt.float32
BF16 = mybir.dt.bfloat16
BF = ml_dtypes.bfloat16
P = 128
NK = IN_F // P  # 32 contraction chunks for GEMM1
# Row blocks: (rows, pieces). Each piece is a [128, 4096] bf16 DRAM slab
# holding 4096/RB chunks of x^T for this block.
BLOCKS = [(512, 4), (256, 2), (256, 2)]
NPIECE = sum(q for _, q in BLOCKS)  # 8 x 1 MiB
PCOLS = 4096  # piece columns (elements per partition line)
ZC = 512  # GEMM2 moving chunk (one PSUM bank of fp32)
NJ = OUT_F // ZC  # 8 output chunks per row tile
NWARM = 10  # HAM warm-up matmuls (~4 us cold — one SHORT window)

_nc_cache = None


def build_nc() -> bass.Bass:
    nc = bacc.Bacc()
    xt_d = nc.declare_dram_parameter("xt", [NPIECE * P, PCOLS], BF16, isOutput=False)
    # (2A)^T, partition-major chunk tiling: [128, 32*16]
    at_d = nc.declare_dram_parameter("at", [P, NK * R], BF16, isOutput=False)
    bb_d = nc.declare_dram_parameter("bb", [R + 1, OUT_F], BF16, isOutput=False)
    out_d = nc.declare_dram_parameter("out", [ROWS, OUT_F], BF16, isOutput=True)

    with TileContext(nc) as tc:
        with (
            tc.tile_pool(name="const", bufs=1) as const,
            tc.tile_pool(name="xin", bufs=3) as xin,
            tc.tile_pool(name="ytp", bufs=2) as ytp,
            tc.tile_pool(name="zrp", bufs=4) as zrp,
            tc.tile_pool(name="ypsum", bufs=1, space="PSUM") as ypsum,
            tc.tile_pool(name="zpsum", bufs=5, space="PSUM") as zpsum,
        ):
            # HAM warm-up: keep the PE busy through one full activity window
            # while the first x piece loads, so real matmuls run at full
            # clock. Reuses the z PSUM rotation — no extra bank.
            wsrc = const.tile([P, ZC], BF16)
            nc.vector.memset(wsrc[:, :], 0.0)
            w_ps = zpsum.tile([P, ZC], FP32, tag="zz")
            for _ in range(NWARM):
                nc.tensor.matmul(
                    w_ps, lhsT=wsrc[:, :P], rhs=wsrc[:, :], start=True, stop=True
                )

            # Const loads on the ACT ring — keep the SP ring free for x.
            at_sb = const.tile([P, NK * R], BF16)
            nc.scalar.dma_start(out=at_sb[:, :], in_=at_d[:, :])
            bb = const.tile([R + 1, OUT_F], BF16)
            nc.scalar.dma_start(out=bb[:, :], in_=bb_d[:, :])

            piece_idx = [0]

            def load_piece():
                i = piece_idx[0]
                piece_idx[0] += 1
                xt_p = xin.tile([P, PCOLS], BF16, tag="x")
                nc.sync.dma_start(
                    out=xt_p[:, :], in_=xt_d[i * P : (i + 1) * P, :]
                )
                return xt_p

            def g1_group(y_ps, piece, q, rb):
                kq = PCOLS // rb  # chunks per piece
                for kk in range(kq):
                    k = q * kq + kk
                    nc.tensor.matmul(
                        y_ps,
                        lhsT=at_sb[:, k * R : (k + 1) * R],
                        rhs=piece[:, kk * rb : (kk + 1) * rb],
                        start=(k == 0),
                        stop=(k == NK - 1),
                    )

            def make_yt(y_ps, rb):
                # Ones-fill the whole tile (engines can't start at partition
                # 16), then overwrite rows 0:16 with y — row 16 keeps 1.0.
                yt_sb = ytp.tile([R + 1, rb], BF16, tag=f"yt{rb}")
                nc.vector.memset(yt_sb[:, :], 1.0)
                nc.scalar.copy(out=yt_sb[0:R, :], in_=y_ps)
                return yt_sb

            row_tile = [0]

            def g2_tile(yt_sb, h):
                rt = row_tile[0]
                row_tile[0] += 1
                zrow = zrp.tile([P, OUT_F], BF16, tag="z")
                for j in range(NJ):
                    z_ps = zpsum.tile([P, ZC], FP32, tag="zz")
                    nc.tensor.matmul(
                        z_ps,
                        lhsT=yt_sb[:, h * P : (h + 1) * P],
                        rhs=bb[:, j * ZC : (j + 1) * ZC],
                        start=True,
                        stop=True,
                    )
                    dst = zrow[:, j * ZC : (j + 1) * ZC]
                    if j % 2 == 0:
                        nc.vector.tensor_copy(out=dst, in_=z_ps)
                    else:
                        nc.scalar.copy(out=dst, in_=z_ps)
                nc.scalar.dma_start(
                    out=out_d[rt * P : (rt + 1) * P, :], in_=zrow[:, :]
                )

            # Block 0 (512 rows): load 4 pieces, GEMM1.
            rb0, nq0 = BLOCKS[0]
            y0 = ypsum.tile([R, rb0], FP32, tag="y0")
            pieces0 = [load_piece() for _ in range(nq0)]
            for q in range(nq0):
                g1_group(y0, pieces0[q], q, rb0)
            yt0 = make_yt(y0, rb0)

            # Interleave block 0's GEMM2 tiles with blocks 1/2's loads and
            # GEMM1 so y(b1)/y(b2) complete right after their last pieces
            # land and the final stores start as early as possible.
            rb1, nq1 = BLOCKS[1]
            rb2, nq2 = BLOCKS[2]
            y1 = ypsum.tile([R, rb1], FP32, tag="y1")
            y2 = ypsum.tile([R, rb2], FP32, tag="y2")
            for q in range(nq1):
                p_ = load_piece()
                g2_tile(yt0, q)
                g1_group(y1, p_, q, rb1)
            yt1 = make_yt(y1, rb1)
            for q in range(nq2):
                p_ = load_piece()
                g2_tile(yt0, nq1 + q)
                g1_group(y2, p_, q, rb2)
            yt2 = make_yt(y2, rb2)

            for h in range(rb1 // P):
                g2_tile(yt1, h)
            for h in range(rb2 // P):
                g2_tile(yt2, h)

    nc.finalize()  # Bacc.finalize runs compile(): wait legalization + reg alloc
    return nc


def make_in_maps(x, lora_A, lora_B, bias):
    x2 = np.asarray(x, dtype=np.float32).reshape(B * S, IN_F)
    # (2A)^T [4096, 16] -> partition-major chunk tiling [128, 32*16]
    at = (np.asarray(lora_A, dtype=np.float32).T * SCALING).astype(BF)
    at = np.ascontiguousarray(
        at.reshape(NK, P, R).transpose(1, 0, 2).reshape(P, NK * R)
    )
    bbh = np.ascontiguousarray(
        np.concatenate(
            [
                np.asarray(lora_B, dtype=np.float32).T,
                np.asarray(bias, dtype=np.float32)[None, :],
            ],
            axis=0,
        ).astype(BF)
    )
    xb = x2.astype(BF)
    maps = []
    for s in np.split(xb, N_CORES, axis=0):
        st = s.T  # x^T [4096, 1024]
        col = 0
        slabs = []
        for rb, nq in BLOCKS:
            kq = PCOLS // rb
            blk = st[:, col : col + rb]  # [4096, rb]
            col += rb
            # chunk k = q*kq + kk -> piece q, partition p, col kk*rb + r
            slabs.append(
                blk.reshape(nq, kq, P, rb)
                .transpose(0, 2, 1, 3)
                .reshape(nq * P, PCOLS)
            )
        xt = np.ascontiguousarray(np.concatenate(slabs, axis=0))
        maps.append({"xt": xt, "at": at, "bb": bbh})
    return maps


def run(inputs: dict, trace: bool = False, **kw):
    global _nc_cache
    if _nc_cache is None:
        _nc_cache = build_nc()
    in_maps = make_in_maps(**inputs)
    res = run_bass_kernel_spmd(
        _nc_cache, in_maps, list(range(N_CORES)), trace=trace, **kw
    )
    out = (
        np.concatenate([res.results[i]["out"] for i in range(N_CORES)], axis=0)
        .astype(np.float32)
        .reshape(B, S, OUT_F)
    )
    return out, res


def kernel(**inputs) -> np.ndarray:
    out, _ = run(inputs)
    return out
